# revision 7
# baseline (speedup 1.0000x reference)
"""Causal self-attention (B=2, T=2048, C=2048, H=16) on 8 TRN2 NeuronCores.

Sharding: data-parallel over batch (2) x tensor-parallel over heads (4 heads
per core). Each core computes, for its batch element b and head group g:
  QKV projection for its heads' columns, causal attention for its 4 heads,
  and a partial output projection (row-sharded W_proj). The host sums the
  4 partial projections per batch element.

Key performance structure (v3):
  - All inputs host-prepacked into partition-major layouts so every DMA
    moves >=4KB contiguous per partition line, issued in first-use order.
  - A warm-up spin of dummy matmuls holds the PE HAM clock-gate at 8/8
    (2.4 GHz) while the first input DMAs land.
  - Software-pipelined chunk schedule: chunk j's QKV projection groups are
    interleaved at ~4us granularity with attention SEGMENTS (4 key-tiles)
    of chunk j-1 and the output projection of chunk j-2. Every cross-engine
    dependency (PSUM->DVE qkt copy, ACT exp backlog, DMA transpose) is a
    full chunk old by the time the PE consumes it, so the PE never waits.
  - The y^T transpose needed by the output projection runs on the DMA xbar
    (dma_start_transpose), not the PE.

Per-core device layouts (fp16 compute / fp32 PSUM accumulation):
  xb   [128, 4, 16, 512]  x^T tiles, chunk-major: [p, tj, c, t]
  wqk  [128, 8, 16, 128]  [p, coltile, c, n]; coltiles 0..3 Q heads, 4..7 K
  wv   [128, 16, 512]     [p, c, (h d)]
  wp   [128, 4, 2048]     [p, h, c]  W_proj rows for this head group
  out  [T, C] fp16 partial projection output

Attention per (head, 512-wide q-chunk): S^T = K_kt^T.T @ Q^T per key tile,
P^T = exp(scale*S^T) (ACT), diagonal masks on DVE, Y[q, d+1] += P^T.T @
[V | ones] accumulated in PSUM (the ones column gives the softmax
denominator), y = Y[:, :d] * (1/Y[:, d]) on DVE, then DMA-transpose into
yt[d, h, t] for the projection.
"""

import os

import numpy as np

N_HEAD = 16
N_EMBD = 2048
B = 2
T = 2048
C = N_EMBD
D = C // N_HEAD  # 128
HPC = N_HEAD // 4  # heads per core = 4
N_CORES = 8
CT = C // 128  # 16 contraction tiles
TT = T // 128  # 16 t tiles
NCH = T // 512  # 4 chunks of 512
NWARM = 32

LAST_EXEC_NS = None

_CACHE = {}


def _build_nc():
    import concourse.bass as bass  # noqa: F401
    import concourse.tile as tile
    from concourse import bacc, mybir

    F32 = mybir.dt.float32
    F16 = mybir.dt.float16
    Exp = mybir.ActivationFunctionType.Exp
    Copy = mybir.ActivationFunctionType.Copy
    SCALE = 1.0 / float(np.sqrt(D))

    nc = bacc.Bacc("TRN2", target_bir_lowering=False, num_devices=N_CORES)

    xb_d = nc.dram_tensor("xb", [128, NCH, CT, 512], F16, kind="ExternalInput")
    wqk_d = nc.dram_tensor("wqk", [128, 8, CT, 128], F16, kind="ExternalInput")
    wv_d = nc.dram_tensor("wv", [128, CT, 512], F16, kind="ExternalInput")
    wp_d = nc.dram_tensor("wp", [128, HPC, C], F16, kind="ExternalInput")
    out_d = nc.dram_tensor("out_part", [T, C], F16, kind="ExternalOutput")

    # Diagonal causal masks, partition-major: [128 k, diag idx, 512 q].
    kk = np.arange(128)[:, None]
    qq = np.arange(512)[None, :]
    masks = np.stack(
        [(qq >= (128 * i + kk)).astype(np.float16) for i in range(4)], axis=1
    )  # [128, 4, 512]
    masks_d = nc.inline_tensor(np.ascontiguousarray(masks), name="diagmasks")

    with tile.TileContext(nc) as tc:
        with (
            tc.tile_pool(name="singles", bufs=1) as singles,
            tc.tile_pool(name="xbp", bufs=3) as xbp,
            tc.tile_pool(name="ptp", bufs=8) as ptp,
            tc.tile_pool(name="ysb", bufs=4) as ysbp,
            tc.tile_pool(name="rp", bufs=4) as rp,
            tc.tile_pool(name="ost", bufs=3) as ostp,
            tc.tile_pool(name="ps", bufs=4, space="PSUM") as ps,
            tc.tile_pool(name="yps", bufs=4, space="PSUM") as yps,
        ):
            # ---- PE warm-up: keep the HAM clock-gate at 8/8 while DMAs land
            warm_w = singles.tile([128, 128], F16, name="warm_w")
            warm_x = singles.tile([128, 512], F16, name="warm_x")
            nc.vector.memset(warm_w, 0.0)
            nc.vector.memset(warm_x, 0.0)
            for i in range(NWARM):
                wps = ps.tile([128, 512], F32, tag="ps", name=f"warm{i}")
                nc.tensor.matmul(wps, warm_w, warm_x, start=True, stop=True)

            # ---- Input DMAs, issued in first-use order on the sync ring ----
            wqk_sb = singles.tile([128, 8, CT, 128], F16, name="wqk_sb")
            wv_sb = singles.tile([128, CT, 512], F16, name="wv_sb")
            wp_sb = singles.tile([128, HPC, C], F16, name="wp_sb")
            mask_sb = singles.tile([128, 4, 512], F16, name="mask_sb")
            xb_t = [None] * NCH

            nc.sync.dma_start(out=wqk_sb[:, 4], in_=wqk_d[:, 4])  # first K group
            xb_t[0] = xbp.tile([128, CT, 512], F16, tag="xb", name="xb0")
            nc.sync.dma_start(out=xb_t[0], in_=xb_d[:, 0])
            for ct in (5, 6, 7):
                nc.sync.dma_start(out=wqk_sb[:, ct], in_=wqk_d[:, ct])
            nc.sync.dma_start(out=wv_sb, in_=wv_d[:, :])
            for ct in (0, 1, 2, 3):
                nc.sync.dma_start(out=wqk_sb[:, ct], in_=wqk_d[:, ct])
            nc.sync.dma_start(out=mask_sb, in_=masks_d[:, :, :])
            nc.sync.dma_start(out=wp_sb, in_=wp_d[:, :])
            xb_t[1] = xbp.tile([128, CT, 512], F16, tag="xb", name="xb1")
            nc.sync.dma_start(out=xb_t[1], in_=xb_d[:, 1])

            # qkt: [d, coltile, t]; coltiles 0..3 = Q heads, 4..7 = K heads
            qkt_sb = singles.tile([128, 8, T], F16)
            # v with a ones column per (kt, head): [kt-tile, head, 129]
            vv_sb = singles.tile([128, TT, HPC, 129], F16)
            # y transposed: [d, head, t]
            yt_sb = singles.tile([128, HPC, T], F16)

            def qkv_group(tj, ct):
                # projection group for coltile ct of chunk tj (N=512, 16 MMs)
                xt = xb_t[tj]
                pq = ps.tile([128, 512], F32, tag="ps", name=f"pq{tj}_{ct}")
                for c in range(CT):
                    nc.tensor.matmul(
                        pq,
                        wqk_sb[:, ct, c, :],
                        xt[:, c, :],
                        start=(c == 0),
                        stop=(c == CT - 1),
                    )
                    if c == 3:
                        flush_pending()
                nc.vector.tensor_copy(
                    out=qkt_sb[:, ct, tj * 512 : (tj + 1) * 512], in_=pq
                )

            def v_group(tj, tt):
                kt = tj * 4 + tt
                xt = xb_t[tj]
                pv = ps.tile([128, 512], F32, tag="ps", name=f"pv{kt}")
                for c in range(CT):
                    nc.tensor.matmul(
                        pv,
                        xt[:, c, tt * 128 : (tt + 1) * 128],
                        wv_sb[:, c, :],
                        start=(c == 0),
                        stop=(c == CT - 1),
                    )
                    if c == 3:
                        flush_pending()
                nc.vector.tensor_copy(
                    out=vv_sb[:, kt, :, 0:128],
                    in_=pv.rearrange("p (h d) -> p h d", h=HPC),
                )
                nc.vector.memset(vv_sb[:, kt, :, 128:129], 1.0)

            y_live = {}  # h -> y_tiles for the attention chunk in flight
            pending = []  # deferred AV-batch thunks, spliced into later PE work

            def flush_pending():
                for f in pending:
                    f()
                pending.clear()

            def attn_seg(j, h, sg, final=False):
                # key-tile segment sg (4 kt) of head h, q-chunk j.
                # S matmuls run 2 kt ahead of the AV batches; the last two AV
                # batches (plus the head finalizer) are deferred into the next
                # PE work unit so the PE never waits on the ACT exp latency.
                if sg == 0:
                    y_live[h] = [
                        yps.tile([128, 129], F32, tag="y", name=f"yt{h}_{j}_{qs}")
                        for qs in range(4)
                    ]
                y_tiles = y_live[h]
                pts = {}

                def s_part(kt):
                    di = kt - 4 * j
                    lo = 128 * di if di > 0 else 0
                    ss = ps.tile([128, 512], F32, tag="ps", name=f"ss{h}{j}{kt}")
                    nc.tensor.matmul(
                        ss[:, lo:],
                        qkt_sb[:, 4 + h, kt * 128 : (kt + 1) * 128],
                        qkt_sb[:, h, j * 512 + lo : (j + 1) * 512],
                        start=True,
                        stop=True,
                    )
                    pt = ptp.tile([128, 512], F16, tag="pt", name=f"pt{h}{j}{kt}")
                    nc.scalar.activation(
                        out=pt[:, lo:], in_=ss[:, lo:], func=Exp, scale=SCALE
                    )
                    if di >= 0:
                        nc.vector.tensor_mul(
                            pt[:, lo : lo + 128],
                            pt[:, lo : lo + 128],
                            mask_sb[:, di, lo : lo + 128],
                        )
                    pts[kt] = pt

                def av_part(kt):
                    di = kt - 4 * j
                    pt = pts.pop(kt)
                    for qs in range(max(0, di), 4):
                        nc.tensor.matmul(
                            y_tiles[qs],
                            pt[:, qs * 128 : (qs + 1) * 128],
                            vv_sb[:, kt, h, :],
                            start=(kt == 0),
                            stop=(kt == 4 * j + qs),
                        )

                k0 = 4 * sg
                s_part(k0)
                s_part(k0 + 1)
                flush_pending()
                av_part(k0)
                s_part(k0 + 2)
                av_part(k0 + 1)
                s_part(k0 + 3)
                pending.append(lambda: av_part(k0 + 2))
                pending.append(lambda: av_part(k0 + 3))
                if final:
                    pending.append(lambda: attn_head_end(j, h))

            def attn_head_end(j, h):
                y_tiles = y_live.pop(h)
                for qs in range(4):
                    yt = y_tiles[qs]
                    r = rp.tile([128, 1], F32, tag="r", name=f"r{h}{j}{qs}")
                    nc.vector.reciprocal(r, yt[:, 128:129])
                    y16 = ysbp.tile([128, 128], F16, tag="y16", name=f"y16_{qs}")
                    nc.vector.tensor_scalar_mul(y16, yt[:, 0:128], r)
                    tglob = (j * 4 + qs) * 128
                    nc.sync.dma_start_transpose(
                        out=yt_sb[:, h, tglob : tglob + 128], in_=y16
                    )

            def proj_tile(tt, fine_store=False, dve_copies=False):
                ot = ostp.tile([128, C], F16, tag="ot", name=f"ot{tt}")
                for cc in range(4):
                    po = ps.tile([128, 512], F32, tag="ps", name=f"po{tt}_{cc}")
                    for hd in range(HPC):
                        nc.tensor.matmul(
                            po,
                            yt_sb[:, hd, tt * 128 : (tt + 1) * 128],
                            wp_sb[:, hd, cc * 512 : (cc + 1) * 512],
                            start=(hd == 0),
                            stop=(hd == HPC - 1),
                        )
                    if cc == 0:
                        flush_pending()
                    if dve_copies or cc % 2 == 0:
                        nc.vector.tensor_copy(
                            out=ot[:, cc * 512 : (cc + 1) * 512], in_=po
                        )
                    else:
                        nc.scalar.activation(
                            out=ot[:, cc * 512 : (cc + 1) * 512], in_=po, func=Copy
                        )
                    if fine_store:
                        nc.sync.dma_start(
                            out=out_d[
                                tt * 128 : (tt + 1) * 128,
                                cc * 512 : (cc + 1) * 512,
                            ],
                            in_=ot[:, cc * 512 : (cc + 1) * 512],
                        )
                    elif cc % 2 == 1:
                        nc.sync.dma_start(
                            out=out_d[
                                tt * 128 : (tt + 1) * 128,
                                (cc - 1) * 512 : (cc + 1) * 512,
                            ],
                            in_=ot[:, (cc - 1) * 512 : (cc + 1) * 512],
                        )

            def chunk_groups(tj):
                for ct in (4, 5, 6, 7):
                    yield ("qkv", ct)
                for tt in range(4):
                    yield ("v", tt)
                for h in range(HPC):
                    yield ("qkv", h)

            # ---- chunk 0: pure QKV ----
            for kind, a in chunk_groups(0):
                (qkv_group if kind == "qkv" else v_group)(0, a)

            # ---- chunks 1..3: QKV(j) x attn(j-1) segments x proj(j-2) ----
            for j in range(1, NCH):
                if j + 1 < NCH:
                    xb_t[j + 1] = xbp.tile(
                        [128, CT, 512], F16, tag="xb", name=f"xb{j + 1}"
                    )
                    nc.sync.dma_start(out=xb_t[j + 1], in_=xb_d[:, j + 1])
                aj = j - 1
                segs = [(h, s) for h in range(HPC) for s in range(aj + 1)]
                si = 0
                for gi, (kind, a) in enumerate(chunk_groups(j)):
                    (qkv_group if kind == "qkv" else v_group)(j, a)
                    want = (gi + 1) * len(segs) // 12
                    while si < want:
                        h, s = segs[si]
                        attn_seg(aj, h, s, final=(s == aj))
                        si += 1
                if j >= 2:
                    for tt in range(4 * (j - 2), 4 * (j - 2) + 4):
                        proj_tile(tt)

            # ---- epilogue: attn(3) per head + proj(2) tile, then proj(3) ----
            aj = NCH - 1
            for h in range(HPC):
                for s in range(aj + 1):
                    attn_seg(aj, h, s, final=(s == aj))
                proj_tile(4 * (NCH - 2) + h, dve_copies=True)
            for tt in range(4 * (NCH - 1), 4 * NCH):
                proj_tile(tt, fine_store=(tt == 4 * NCH - 1), dve_copies=True)
            flush_pending()

    nc.compile()
    return nc


def _get_nc():
    if "nc" not in _CACHE:
        _CACHE["nc"] = _build_nc()
    return _CACHE["nc"]


def kernel(x, W_attn, W_proj):
    global LAST_EXEC_NS
    from concourse.bass_utils import run_bass_kernel_spmd

    x = np.asarray(x)
    W_attn = np.asarray(W_attn)
    W_proj = np.asarray(W_proj)

    in_maps = []
    for core in range(N_CORES):
        b, g = divmod(core, 4)
        heads = range(4 * g, 4 * g + 4)
        # x^T tiles, chunk-major: [p, tj, c, t]
        xb = (
            np.ascontiguousarray(x[b].T)
            .astype(np.float16)
            .reshape(CT, 128, NCH, 512)
            .transpose(1, 2, 0, 3)
        )
        wqk = np.concatenate(
            [W_attn[:, h * D : (h + 1) * D] for h in heads]
            + [W_attn[:, C + h * D : C + (h + 1) * D] for h in heads],
            axis=1,
        ).astype(np.float16)  # [C, 1024]
        wqk = wqk.reshape(CT, 128, 8, 128).transpose(1, 2, 0, 3)  # [p, ct, c, n]
        wv = np.concatenate(
            [W_attn[:, 2 * C + h * D : 2 * C + (h + 1) * D] for h in heads], axis=1
        ).astype(np.float16)  # [C, 512]
        wv = wv.reshape(CT, 128, 512).transpose(1, 0, 2)  # [p, c, n]
        wp = W_proj[4 * g * D : 4 * (g + 1) * D, :].astype(np.float16)  # [512, C]
        wp = wp.reshape(HPC, 128, C).transpose(1, 0, 2)  # [p, h, c]
        in_maps.append(
            {
                "xb": np.ascontiguousarray(xb),
                "wqk": np.ascontiguousarray(wqk),
                "wv": np.ascontiguousarray(wv),
                "wp": np.ascontiguousarray(wp),
            }
        )

    nc = _get_nc()
    res = run_bass_kernel_spmd(
        nc,
        in_maps,
        list(range(N_CORES)),
        trace=bool(os.environ.get("KERNEL_TRACE")),
    )
    LAST_EXEC_NS = res.exec_time_ns

    out = np.zeros((B, T, C), dtype=np.float32)
    for core in range(N_CORES):
        b = core // 4
        out[b] += res.results[core]["out_part"].astype(np.float32)
    return out


# revision 10
# speedup vs baseline: 1.0251x; 1.0251x over previous
"""Causal self-attention (B=2, T=2048, C=2048, H=16) on 8 TRN2 NeuronCores.

Sharding: data-parallel over batch (2) x tensor-parallel over heads (4 heads
per core). Each core computes, for its batch element b and head group g:
  QKV projection for its heads' columns, causal attention for its 4 heads,
  and a partial output projection (row-sharded W_proj). The host sums the
  4 partial projections per batch element.

Key performance structure (v3):
  - All inputs host-prepacked into partition-major layouts so every DMA
    moves >=4KB contiguous per partition line, issued in first-use order.
  - A warm-up spin of dummy matmuls holds the PE HAM clock-gate at 8/8
    (2.4 GHz) while the first input DMAs land.
  - Software-pipelined chunk schedule: chunk j's QKV projection groups are
    interleaved at ~4us granularity with attention SEGMENTS (4 key-tiles)
    of chunk j-1 and the output projection of chunk j-2. Every cross-engine
    dependency (PSUM->DVE qkt copy, ACT exp backlog, DMA transpose) is a
    full chunk old by the time the PE consumes it, so the PE never waits.
  - The y^T transpose needed by the output projection runs on the DMA xbar
    (dma_start_transpose), not the PE.

Per-core device layouts (fp16 compute / fp32 PSUM accumulation):
  xb   [128, 4, 16, 512]  x^T tiles, chunk-major: [p, tj, c, t]
  wqk  [128, 8, 16, 128]  [p, coltile, c, n]; coltiles 0..3 Q heads, 4..7 K
  wv   [128, 16, 512]     [p, c, (h d)]
  wp   [128, 4, 2048]     [p, h, c]  W_proj rows for this head group
  out  [T, C] fp16 partial projection output

Attention per (head, 512-wide q-chunk): S^T = K_kt^T.T @ Q^T per key tile,
P^T = exp(scale*S^T) (ACT), diagonal masks on DVE, Y[q, d+1] += P^T.T @
[V | ones] accumulated in PSUM (the ones column gives the softmax
denominator), y = Y[:, :d] * (1/Y[:, d]) on DVE, then DMA-transpose into
yt[d, h, t] for the projection.
"""

import os

import numpy as np

N_HEAD = 16
N_EMBD = 2048
B = 2
T = 2048
C = N_EMBD
D = C // N_HEAD  # 128
HPC = N_HEAD // 4  # heads per core = 4
N_CORES = 8
CT = C // 128  # 16 contraction tiles
TT = T // 128  # 16 t tiles
NCH = T // 512  # 4 chunks of 512
NWARM = 32

LAST_EXEC_NS = None

_CACHE = {}


def _build_nc():
    import concourse.bass as bass  # noqa: F401
    import concourse.tile as tile
    from concourse import bacc, mybir

    F32 = mybir.dt.float32
    F16 = mybir.dt.float16
    Exp = mybir.ActivationFunctionType.Exp
    Copy = mybir.ActivationFunctionType.Copy
    SCALE = 1.0 / float(np.sqrt(D))

    nc = bacc.Bacc("TRN2", target_bir_lowering=False, num_devices=N_CORES)

    xb_d = nc.dram_tensor("xb", [128, NCH, CT, 512], F16, kind="ExternalInput")
    wqk_d = nc.dram_tensor("wqk", [128, 8, CT, 128], F16, kind="ExternalInput")
    wv_d = nc.dram_tensor("wv", [128, CT, 512], F16, kind="ExternalInput")
    wp_d = nc.dram_tensor("wp", [128, HPC, C], F16, kind="ExternalInput")
    out_d = nc.dram_tensor("out_part", [T, C], F16, kind="ExternalOutput")

    # Diagonal causal masks, partition-major: [128 k, diag idx, 512 q].
    kk = np.arange(128)[:, None]
    qq = np.arange(512)[None, :]
    masks = np.stack(
        [(qq >= (128 * i + kk)).astype(np.float16) for i in range(4)], axis=1
    )  # [128, 4, 512]
    masks_d = nc.inline_tensor(np.ascontiguousarray(masks), name="diagmasks")

    with tile.TileContext(nc) as tc:
        with (
            tc.tile_pool(name="singles", bufs=1) as singles,
            tc.tile_pool(name="xbp", bufs=3) as xbp,
            tc.tile_pool(name="ptp", bufs=8) as ptp,
            tc.tile_pool(name="ysb", bufs=4) as ysbp,
            tc.tile_pool(name="rp", bufs=4) as rp,
            tc.tile_pool(name="ost", bufs=3) as ostp,
            tc.tile_pool(name="ps", bufs=6, space="PSUM") as ps,
            tc.tile_pool(name="yps", bufs=2, space="PSUM") as yps,
        ):
            # ---- PE warm-up: keep the HAM clock-gate at 8/8 while DMAs land
            warm_w = singles.tile([128, 128], F16, name="warm_w")
            warm_x = singles.tile([128, 512], F16, name="warm_x")
            nc.vector.memset(warm_w, 0.0)
            nc.vector.memset(warm_x, 0.0)
            for i in range(NWARM):
                wps = ps.tile([128, 512], F32, tag="ps", name=f"warm{i}")
                nc.tensor.matmul(wps, warm_w, warm_x, start=True, stop=True)

            # ---- Input DMAs, issued in first-use order on the sync ring ----
            wqk_sb = singles.tile([128, 8, CT, 128], F16, name="wqk_sb")
            wv_sb = singles.tile([128, CT, 512], F16, name="wv_sb")
            wp_sb = singles.tile([128, HPC, C], F16, name="wp_sb")
            mask_sb = singles.tile([128, 4, 512], F16, name="mask_sb")
            xb_t = [None] * NCH

            nc.sync.dma_start(out=wqk_sb[:, 4], in_=wqk_d[:, 4])  # first K group
            xb_t[0] = xbp.tile([128, CT, 512], F16, tag="xb", name="xb0")
            nc.sync.dma_start(out=xb_t[0], in_=xb_d[:, 0])
            for ct in (5, 6, 7):
                nc.sync.dma_start(out=wqk_sb[:, ct], in_=wqk_d[:, ct])
            nc.sync.dma_start(out=wv_sb, in_=wv_d[:, :])
            for ct in (0, 1, 2, 3):
                nc.sync.dma_start(out=wqk_sb[:, ct], in_=wqk_d[:, ct])
            nc.sync.dma_start(out=mask_sb, in_=masks_d[:, :, :])
            nc.sync.dma_start(out=wp_sb, in_=wp_d[:, :])
            xb_t[1] = xbp.tile([128, CT, 512], F16, tag="xb", name="xb1")
            nc.sync.dma_start(out=xb_t[1], in_=xb_d[:, 1])

            # qkt: [d, coltile, t]; coltiles 0..3 = Q heads, 4..7 = K heads
            qkt_sb = singles.tile([128, 8, T], F16)
            # v with a ones column per (kt, head): [kt-tile, head, 129]
            vv_sb = singles.tile([128, TT, HPC, 129], F16)
            # y transposed: [d, head, t]
            yt_sb = singles.tile([128, HPC, T], F16)

            def qkv_group(tj, ct):
                # projection group for coltile ct of chunk tj (N=512, 16 MMs)
                xt = xb_t[tj]
                pq = ps.tile([128, 512], F32, tag="ps", name=f"pq{tj}_{ct}")
                for c in range(CT):
                    nc.tensor.matmul(
                        pq,
                        wqk_sb[:, ct, c, :],
                        xt[:, c, :],
                        start=(c == 0),
                        stop=(c == CT - 1),
                    )
                nc.vector.tensor_copy(
                    out=qkt_sb[:, ct, tj * 512 : (tj + 1) * 512], in_=pq
                )

            def v_group(tj, tt):
                kt = tj * 4 + tt
                xt = xb_t[tj]
                pv = ps.tile([128, 512], F32, tag="ps", name=f"pv{kt}")
                for c in range(CT):
                    nc.tensor.matmul(
                        pv,
                        xt[:, c, tt * 128 : (tt + 1) * 128],
                        wv_sb[:, c, :],
                        start=(c == 0),
                        stop=(c == CT - 1),
                    )
                nc.vector.tensor_copy(
                    out=vv_sb[:, kt, :, 0:128],
                    in_=pv.rearrange("p (h d) -> p h d", h=HPC),
                )
                nc.vector.memset(vv_sb[:, kt, :, 128:129], 1.0)

            y_live = {}  # h -> y_tiles for the attention chunk in flight

            def attn_seg(j, h, sg, final=False):
                # key-tile segment sg (4 kt) of head h, q-chunk j
                if sg == 0:
                    pairs = [
                        yps.tile([128, 258], F32, tag="y", name=f"yp{h}_{j}_{q}")
                        for q in range(2)
                    ]
                    y_live[h] = [(pairs[qs // 2], (qs % 2) * 129) for qs in range(4)]
                y_tiles = y_live[h]
                for kt in range(4 * sg, 4 * sg + 4):
                    di = kt - 4 * j
                    lo = 128 * di if di > 0 else 0
                    ss = ps.tile([128, 512], F32, tag="ps", name=f"ss{h}{j}{kt}")
                    nc.tensor.matmul(
                        ss[:, lo:],
                        qkt_sb[:, 4 + h, kt * 128 : (kt + 1) * 128],
                        qkt_sb[:, h, j * 512 + lo : (j + 1) * 512],
                        start=True,
                        stop=True,
                    )
                    pt = ptp.tile([128, 512], F16, tag="pt", name=f"pt{h}{j}{kt}")
                    nc.scalar.activation(
                        out=pt[:, lo:], in_=ss[:, lo:], func=Exp, scale=SCALE
                    )
                    if di >= 0:
                        nc.vector.tensor_mul(
                            pt[:, lo : lo + 128],
                            pt[:, lo : lo + 128],
                            mask_sb[:, di, lo : lo + 128],
                        )
                    for qs in range(max(0, di), 4):
                        # paired accumulators share a PSUM bank; start=True
                        # clears the whole bank, so only the off==0 group may
                        # use it (the clear also zeroes its bank-mate, which
                        # then accumulates from zero with start=False).
                        yp, off = y_tiles[qs]
                        nc.tensor.matmul(
                            yp[:, off : off + 129],
                            pt[:, qs * 128 : (qs + 1) * 128],
                            vv_sb[:, kt, h, :],
                            start=(kt == 0 and off == 0),
                            stop=(kt == 4 * j + qs),
                            skip_group_check=(off != 0),
                        )
                if final:
                    attn_head_end(j, h)

            def attn_head_end(j, h):
                y_tiles = y_live.pop(h)
                for qs in range(4):
                    yp, off = y_tiles[qs]
                    r = rp.tile([128, 1], F32, tag="r", name=f"r{h}{j}{qs}")
                    nc.vector.reciprocal(r, yp[:, off + 128 : off + 129])
                    y16 = ysbp.tile([128, 128], F16, tag="y16", name=f"y16_{qs}")
                    nc.vector.tensor_scalar_mul(y16, yp[:, off : off + 128], r)
                    tglob = (j * 4 + qs) * 128
                    nc.sync.dma_start_transpose(
                        out=yt_sb[:, h, tglob : tglob + 128], in_=y16
                    )

            def proj_tile(tt, fine_store=False, dve_copies=False):
                ot = ostp.tile([128, C], F16, tag="ot", name=f"ot{tt}")
                for cc in range(4):
                    po = ps.tile([128, 512], F32, tag="ps", name=f"po{tt}_{cc}")
                    for hd in range(HPC):
                        nc.tensor.matmul(
                            po,
                            yt_sb[:, hd, tt * 128 : (tt + 1) * 128],
                            wp_sb[:, hd, cc * 512 : (cc + 1) * 512],
                            start=(hd == 0),
                            stop=(hd == HPC - 1),
                        )
                    if dve_copies or cc % 2 == 0:
                        nc.vector.tensor_copy(
                            out=ot[:, cc * 512 : (cc + 1) * 512], in_=po
                        )
                    else:
                        nc.scalar.activation(
                            out=ot[:, cc * 512 : (cc + 1) * 512], in_=po, func=Copy
                        )
                    if fine_store:
                        nc.sync.dma_start(
                            out=out_d[
                                tt * 128 : (tt + 1) * 128,
                                cc * 512 : (cc + 1) * 512,
                            ],
                            in_=ot[:, cc * 512 : (cc + 1) * 512],
                        )
                    elif cc % 2 == 1:
                        nc.sync.dma_start(
                            out=out_d[
                                tt * 128 : (tt + 1) * 128,
                                (cc - 1) * 512 : (cc + 1) * 512,
                            ],
                            in_=ot[:, (cc - 1) * 512 : (cc + 1) * 512],
                        )

            def chunk_groups(tj):
                for ct in (4, 5, 6, 7):
                    yield ("qkv", ct)
                for tt in range(4):
                    yield ("v", tt)
                for h in range(HPC):
                    yield ("qkv", h)

            # ---- chunk 0: pure QKV ----
            for kind, a in chunk_groups(0):
                (qkv_group if kind == "qkv" else v_group)(0, a)

            # ---- chunks 1..3: QKV(j) x attn(j-1) segments x proj(j-2) ----
            for j in range(1, NCH):
                if j + 1 < NCH:
                    xb_t[j + 1] = xbp.tile(
                        [128, CT, 512], F16, tag="xb", name=f"xb{j + 1}"
                    )
                    nc.sync.dma_start(out=xb_t[j + 1], in_=xb_d[:, j + 1])
                aj = j - 1
                segs = [(h, s) for h in range(HPC) for s in range(aj + 1)]
                si = 0
                for gi, (kind, a) in enumerate(chunk_groups(j)):
                    (qkv_group if kind == "qkv" else v_group)(j, a)
                    want = (gi + 1) * len(segs) // 12
                    while si < want:
                        h, s = segs[si]
                        attn_seg(aj, h, s, final=(s == aj))
                        si += 1
                if j >= 2:
                    for tt in range(4 * (j - 2), 4 * (j - 2) + 4):
                        proj_tile(tt)

            # ---- epilogue: attn(3) per head + proj(2) tile, then proj(3) ----
            aj = NCH - 1
            for h in range(HPC):
                for s in range(aj + 1):
                    attn_seg(aj, h, s, final=(s == aj))
                proj_tile(4 * (NCH - 2) + h, dve_copies=True)
            for tt in range(4 * (NCH - 1), 4 * NCH):
                proj_tile(tt, fine_store=(tt == 4 * NCH - 1))

    nc.compile()
    return nc


def _get_nc():
    if "nc" not in _CACHE:
        _CACHE["nc"] = _build_nc()
    return _CACHE["nc"]


def kernel(x, W_attn, W_proj):
    global LAST_EXEC_NS
    from concourse.bass_utils import run_bass_kernel_spmd

    x = np.asarray(x)
    W_attn = np.asarray(W_attn)
    W_proj = np.asarray(W_proj)

    in_maps = []
    for core in range(N_CORES):
        b, g = divmod(core, 4)
        heads = range(4 * g, 4 * g + 4)
        # x^T tiles, chunk-major: [p, tj, c, t]
        xb = (
            np.ascontiguousarray(x[b].T)
            .astype(np.float16)
            .reshape(CT, 128, NCH, 512)
            .transpose(1, 2, 0, 3)
        )
        wqk = np.concatenate(
            [W_attn[:, h * D : (h + 1) * D] for h in heads]
            + [W_attn[:, C + h * D : C + (h + 1) * D] for h in heads],
            axis=1,
        ).astype(np.float16)  # [C, 1024]
        wqk = wqk.reshape(CT, 128, 8, 128).transpose(1, 2, 0, 3)  # [p, ct, c, n]
        wv = np.concatenate(
            [W_attn[:, 2 * C + h * D : 2 * C + (h + 1) * D] for h in heads], axis=1
        ).astype(np.float16)  # [C, 512]
        wv = wv.reshape(CT, 128, 512).transpose(1, 0, 2)  # [p, c, n]
        wp = W_proj[4 * g * D : 4 * (g + 1) * D, :].astype(np.float16)  # [512, C]
        wp = wp.reshape(HPC, 128, C).transpose(1, 0, 2)  # [p, h, c]
        in_maps.append(
            {
                "xb": np.ascontiguousarray(xb),
                "wqk": np.ascontiguousarray(wqk),
                "wv": np.ascontiguousarray(wv),
                "wp": np.ascontiguousarray(wp),
            }
        )

    nc = _get_nc()
    res = run_bass_kernel_spmd(
        nc,
        in_maps,
        list(range(N_CORES)),
        trace=bool(os.environ.get("KERNEL_TRACE")),
    )
    LAST_EXEC_NS = res.exec_time_ns

    out = np.zeros((B, T, C), dtype=np.float32)
    for core in range(N_CORES):
        b = core // 4
        out[b] += res.results[core]["out_part"].astype(np.float32)
    return out
